# revision 1
# baseline (speedup 1.0000x reference)
"""Trainium2 Bass kernel for nn_DecoderPromptLayerWithNMR.

Sharding: 8 fully-independent shards, core = (batch b in 0..3, query-half j in 0..1).
Each core computes the full layer for 512 target queries of one batch element.
No collectives. Feature-major attention with softmax-denominator via ones-column.
"""
import sys
import os

sys.path.insert(0, "/opt/trn_rl_repo")

import numpy as np
import ml_dtypes

import concourse.bass as bass
from concourse import bacc, mybir
from concourse.tile import TileContext

F32 = mybir.dt.float32
BF16 = mybir.dt.bfloat16
AF = mybir.ActivationFunctionType
OP = mybir.AluOpType

# Problem dims
D = 1024
H = 16
DH = 64
ROT = 32
FFN = 4096
B, T, M, N = 4, 1024, 128, 64
PREF = M + N            # 192
PAD_PREF = 256          # padded prefix (64 zero rows, masked)
LP = PAD_PREF + T       # 1280 padded key length
KK = LP // 128          # 10 key tiles
TQ = T // 2             # 512 queries per core
QT = TQ // 128          # 4 query token tiles
DT = D // 128           # 8 feature tiles
FT = FFN // 128         # 32 ffn tiles
EPS = 1e-5
# per-key-tile active query columns (q-blocks sorted by descending causal
# extent per core; envelope over both core parities)
NACT = [512, 512, 512, 448, 384, 320, 256, 192, 128, 64]
# 64-query blocks, one of each causal extent per parity (identical multisets)
QPERM = {0: [14, 12, 10, 8, 6, 4, 2, 0], 1: [15, 13, 11, 9, 7, 5, 3, 1]}
VS = DH + 2             # V row stride (64 V + 1 ones + 1 pad, keeps 4B align)


def build_kernel():
    nc = bacc.Bacc(None, target_bir_lowering=False)

    xin = nc.declare_dram_parameter("xin", [LP, D], F32, isOutput=False)
    xq = nc.declare_dram_parameter("xq", [TQ, D], F32, isOutput=False)
    # packed weights: w*[p, k, n] = W[k*128+p, n]
    wq = nc.declare_dram_parameter("wq", [128, DT, D], BF16, isOutput=False)
    wk = nc.declare_dram_parameter("wk", [128, DT, D], BF16, isOutput=False)
    wv = nc.declare_dram_parameter("wv", [128, DT, D], BF16, isOutput=False)
    wo = nc.declare_dram_parameter("wo", [128, DT, D], BF16, isOutput=False)
    # w1p[m, p, k, c] = W1[k*128+p, m*128+c]; w2p[m, p, k, c] = W2[k*128+p, m*128+c]
    w1 = nc.declare_dram_parameter("w1", [FT, 128, DT, 128], BF16, isOutput=False)
    w2 = nc.declare_dram_parameter("w2", [DT, 128, FT, 128], BF16, isOutput=False)
    bq = nc.declare_dram_parameter("bq", [128, DT], F32, isOutput=False)
    bk = nc.declare_dram_parameter("bk", [128, DT], F32, isOutput=False)
    bv_r = nc.declare_dram_parameter("bv_r", [1, D], BF16, isOutput=False)
    bo = nc.declare_dram_parameter("bo", [128, DT], F32, isOutput=False)
    b1 = nc.declare_dram_parameter("b1", [128, FT], F32, isOutput=False)
    b2 = nc.declare_dram_parameter("b2", [128, DT], F32, isOutput=False)
    cosq = nc.declare_dram_parameter("cosq", [128, TQ], BF16, isOutput=False)
    sinq = nc.declare_dram_parameter("sinq", [128, TQ], BF16, isOutput=False)
    cosk = nc.declare_dram_parameter("cosk", [128, LP], BF16, isOutput=False)
    sink = nc.declare_dram_parameter("sink", [128, LP], BF16, isOutput=False)
    maskt = nc.declare_dram_parameter("maskt", [128, KK, TQ], BF16, isOutput=False)
    yout = nc.declare_dram_parameter("y", [TQ, D], F32, isOutput=True)

    # DRAM scratch for transpose bounces

    with TileContext(nc) as tc:
        with tc.tile_pool(name="persist", bufs=1) as persist, \
             tc.tile_pool(name="stats", bufs=10) as statsp:

            eps_t = persist.tile([128, 1], F32)
            nc.vector.memset(eps_t, EPS)
            ones_t = persist.tile([1, 128], BF16)
            nc.vector.memset(ones_t, 1.0)
            ident = persist.tile([128, 128], BF16)
            from concourse.masks import make_identity
            make_identity(nc, ident[:])

            v_sb = persist.tile([128, KK, H, VS], BF16)  # V token-major + ones
            attnT = persist.tile([128, DT, TQ], BF16)    # normalized attn^T
            x1 = persist.tile([128, QT, D], F32)         # attn-out + residual
            mask_sb = persist.tile([128, KK, TQ], BF16)

            nc.vector.memset(v_sb[:, :, :, DH:DH + 1], 1.0)

            def ln_stats(src_ap):
                """src_ap: [128,1024] fp32 -> (mean, rstd) [128,1] each."""
                st = statsp.tile([128, 2, 6], F32, tag="bn")
                nc.vector.bn_stats(out=st[:, 0, :], in_=src_ap[:, 0:512])
                nc.vector.bn_stats(out=st[:, 1, :], in_=src_ap[:, 512:1024])
                mv = statsp.tile([128, 2], F32, tag="mv")
                nc.vector.bn_aggr(out=mv[:], in_=st[:])
                rstd = statsp.tile([128, 1], F32, tag="rstd")
                nc.scalar.activation(out=rstd[:], in_=mv[:, 1:2],
                                     func=AF.Sqrt, bias=eps_t[:], scale=1.0)
                nc.vector.reciprocal(out=rstd[:], in_=rstd[:])
                return mv[:, 0:1], rstd

            def ln_apply(dst_ap, src_ap, mean, rstd):
                nmr_t = statsp.tile([128, 1], F32, tag="nmr")
                nc.vector.tensor_tensor(nmr_t[:], mean, rstd[:], OP.mult)
                nc.vector.tensor_scalar_mul(nmr_t[:], nmr_t[:], -1.0)
                nc.scalar.activation(out=dst_ap, in_=src_ap, func=AF.Identity,
                                     bias=nmr_t[:], scale=rstd[:])

            with tc.tile_pool(name="pa", bufs=1) as pa:
                # xaT split in two halves so PE work can start before LN1 ends
                xaT_a = pa.tile([128, DT, 512], BF16)   # tokens 0..511
                xaT_b = pa.tile([128, DT, LP - 512], BF16)  # tokens 512..1279
                xqT = pa.tile([128, DT, TQ], BF16)      # LN1(xq)^T
                cosq_sb = pa.tile([128, TQ], BF16)
                sinq_sb = pa.tile([128, TQ], BF16)
                cosk_sb = pa.tile([128, LP], BF16)
                sink_sb = pa.tile([128, LP], BF16)


                def xa_slice(k, t0, t1):
                    # feature-major LN1(xin) tokens [t0, t1) from the split tiles
                    if t1 <= 512:
                        return xaT_a[:, k, t0:t1]
                    return xaT_b[:, k, t0 - 512:t1 - 512]

                from contextlib import ExitStack
                _wstack = ExitStack()
                wp = _wstack.enter_context(tc.tile_pool(name="w_p", bufs=1))
                wq_sb = wp.tile([128, DT, D], BF16, tag="wq")
                wk_sb = wp.tile([128, DT, D], BF16, tag="wk")
                wv_sb = wp.tile([128, DT, D], BF16, tag="wv")
                bias_sb = wp.tile([128, 2 * DT], F32, tag="bias")
                bvr_sb = wp.tile([1, D], BF16, tag="bvr")

                # ---------- Phase 1+2: LN1 + transposes, xq first ------------
                with tc.tile_pool(name="ln", bufs=6) as lnp:

                    with tc.tile_pool(name="tr_ps", bufs=8, space="PSUM") as trps:
                        def ln_rows_t(src, t0, t1, dstT, dt0):
                            # LN + PE-transpose into feature-major dstT;
                            # half-column loads so bn_stats starts earlier
                            for t in range(t0, t1):
                                x_t = lnp.tile([128, D], F32, tag="ln_in")
                                nc.sync.dma_start(
                                    x_t[:, 0:512],
                                    src[t * 128:(t + 1) * 128, 0:512])
                                nc.sync.dma_start(
                                    x_t[:, 512:1024],
                                    src[t * 128:(t + 1) * 128, 512:1024])
                                mean, rstd = ln_stats(x_t[:])
                                xa_t = lnp.tile([128, D], BF16, tag="ln_out")
                                ln_apply(xa_t[:], x_t[:], mean, rstd)
                                d = t - dt0
                                for f in range(DT):
                                    tp = trps.tile([128, 128], BF16, tag="tr")
                                    nc.tensor.transpose(
                                        tp[:], xa_t[:, f * 128:(f + 1) * 128],
                                        ident[:])
                                    nc.any.tensor_copy(
                                        dstT[:, f, d * 128:(d + 1) * 128], tp[:])

                        ln_rows_t(xq, 0, 1, xqT, 0)
                        nc.sync.dma_start(wq_sb[:], wq[:])
                        nc.sync.dma_start(bias_sb[:, 0:DT], bk[:])
                        nc.sync.dma_start(bias_sb[:, DT:2 * DT], bq[:])
                        ln_rows_t(xq, 1, QT, xqT, 0)
                        nc.sync.dma_start(cosq_sb[:], cosq[:])
                        nc.sync.dma_start(sinq_sb[:], sinq[:])
                        nc.sync.dma_start(cosk_sb[:], cosk[:])
                        nc.sync.dma_start(sink_sb[:], sink[:])
                        ln_rows_t(xin, 0, 4, xaT_a, 0)
                        nc.sync.dma_start(wv_sb[:], wv[:])
                        nc.sync.dma_start(wk_sb[:], wk[:])
                        nc.sync.dma_start(bvr_sb[:], bv_r[:])
                        ln_rows_t(xin, 4, KK, xaT_b, 4)

                def rope(dst_ap, ntok, cos_sb, sin_sb, pool):
                    # sin table carries the rotate-half sign; shifts via DMA
                    rot = pool.tile([128, ntok], BF16, tag="rot")
                    nc.vector.memset(rot[32:64, :], 0.0)
                    nc.vector.memset(rot[96:128, :], 0.0)
                    nc.sync.dma_start(rot[0:16, :], dst_ap[16:32, :])
                    nc.sync.dma_start(rot[16:32, :], dst_ap[0:16, :])
                    nc.sync.dma_start(rot[64:80, :], dst_ap[80:96, :])
                    nc.sync.dma_start(rot[80:96, :], dst_ap[64:80, :])
                    nc.vector.tensor_tensor(rot[:], rot[:], sin_sb[:, :ntok], OP.mult)
                    nc.vector.tensor_tensor(dst_ap, dst_ap, cos_sb[:, :ntok], OP.mult)
                    nc.vector.tensor_tensor(dst_ap, dst_ap, rot[:], OP.add)

                # ---------- Phase 3a: Q proj, then V (token-major) -----------
                qTs = []
                with tc.tile_pool(name="v_ps", bufs=3, space="PSUM") as vps, \
                     tc.tile_pool(name="q_psp", bufs=2, space="PSUM") as qps:
                    for hg in range(H // 2):
                        qT_h = pa.tile([128, TQ], BF16, tag=f"qT{hg}")
                        ps = qps.tile([128, 512], F32, tag="q_ps")
                        for k in range(DT):
                            nc.tensor.matmul(
                                ps[:], lhsT=wq_sb[:, k, hg * 128:(hg + 1) * 128],
                                rhs=xqT[:, k, :],
                                start=(k == 0), stop=(k == DT - 1))
                        nc.vector.tensor_scalar_add(
                            qT_h[:], ps[:], bias_sb[:, DT + hg:DT + hg + 1])
                        rope(qT_h[:], TQ, cosq_sb, sinq_sb, statsp)
                        qTs.append(qT_h)
                    for kk in range(KK):
                        ps = vps.tile([128, 1024], F32, tag="v_ps")
                        for c0 in (0, 512):
                            for k in range(DT):
                                nc.tensor.matmul(
                                    ps[:, c0:c0 + 512],
                                    lhsT=xa_slice(k, kk * 128, (kk + 1) * 128),
                                    rhs=wv_sb[:, k, c0:c0 + 512],
                                    start=(k == 0), stop=False)
                            nc.tensor.matmul(
                                ps[:, c0:c0 + 512],
                                lhsT=ones_t[:],
                                rhs=bvr_sb[0:1, c0:c0 + 512],
                                start=False, stop=True)
                        nc.vector.tensor_copy(
                            out=v_sb[:, kk, :, 0:DH],
                            in_=ps[:].rearrange("p (h d) -> p h d", h=H))

                # ---------- Phase 3b/4: per head-group K proj + attention ----
                with tc.tile_pool(name="kq_sb", bufs=3) as kqsb, \
                     tc.tile_pool(name="kq_ps", bufs=2, space="PSUM") as pps, \
                     tc.tile_pool(name="sc_ps", bufs=2, space="PSUM") as scps, \
                     tc.tile_pool(name="at_ps", bufs=2, space="PSUM") as atps, \
                     tc.tile_pool(name="att_tmp", bufs=4) as atp:
                    nc.sync.dma_start(mask_sb[:], maskt[:])
                    for hg in range(H // 2):
                        qT_h = qTs[hg]
                        # K^T tile for heads 2hg, 2hg+1
                        kT_h = kqsb.tile([128, LP], BF16, tag="kT")
                        for c0 in range(0, LP, 512):
                            cw = min(512, LP - c0)
                            ps = pps.tile([128, 512], F32, tag="kq_ps")
                            for k in range(DT):
                                nc.tensor.matmul(
                                    ps[:, :cw],
                                    lhsT=wk_sb[:, k, hg * 128:(hg + 1) * 128],
                                    rhs=xa_slice(k, c0, c0 + cw),
                                    start=(k == 0), stop=(k == DT - 1))
                            nc.vector.tensor_scalar_add(
                                kT_h[:, c0:c0 + cw], ps[:, :cw],
                                bias_sb[:, hg:hg + 1])
                        rope(kT_h[:], LP, cosk_sb, sink_sb, atp)

                        # attention for this head pair
                        ap0 = atps.tile([DH + 1, TQ], F32, tag="ap")
                        ap1 = atps.tile([DH + 1, TQ], F32, tag="ap")
                        aps = (ap0, ap1)
                        for kk in range(KK):
                            na = NACT[kk]
                            sp = scps.tile([128, 1024], F32, tag="scores")
                            for i in range(2):
                                r0 = 64 * i
                                nc.tensor.matmul(
                                    sp[:, i * 512:i * 512 + na],
                                    lhsT=kT_h[r0:r0 + 64, kk * 128:(kk + 1) * 128],
                                    rhs=qT_h[r0:r0 + 64, 0:na],
                                    start=True, stop=True)
                            pexp = atp.tile([128, 2, 512], BF16, tag="pexp")
                            if na == TQ:
                                nc.scalar.activation(
                                    out=pexp[:].rearrange("p a b -> p (a b)"),
                                    in_=sp[:], func=AF.Exp, scale=0.125)
                            else:
                                for i in range(2):
                                    nc.scalar.activation(
                                        out=pexp[:, i, 0:na],
                                        in_=sp[:, i * 512:i * 512 + na],
                                        func=AF.Exp, scale=0.125)
                            nc.vector.tensor_tensor(
                                pexp[:, :, 0:na],
                                pexp[:, :, 0:na],
                                mask_sb[:, kk, None, 0:na].to_broadcast([128, 2, na]),
                                OP.mult)
                            for i in range(2):
                                nc.tensor.matmul(
                                    aps[i][:, 0:na],
                                    lhsT=v_sb[:, kk, 2 * hg + i, 0:DH + 1],
                                    rhs=pexp[:, i, 0:na],
                                    start=(kk == 0), stop=(kk == KK - 1))
                        for i in range(2):
                            r_sb = atp.tile([1, TQ], F32, tag="recip")
                            nc.vector.reciprocal(r_sb[:], aps[i][DH:DH + 1, :])
                            bsb = atp.tile([64, TQ], F32, tag="bcs")
                            nc.gpsimd.partition_broadcast(bsb[:], r_sb[:])
                            nc.vector.tensor_tensor(
                                attnT[64 * i:64 * i + 64, hg, :],
                                aps[i][0:DH, :], bsb[:], OP.mult)

                _wstack.close()

            # ---------- Phase 5: out-proj + residual -------------------------
            with tc.tile_pool(name="wo_p", bufs=1) as wop, \
                 tc.tile_pool(name="o_ps", bufs=4, space="PSUM") as ops, \
                 tc.tile_pool(name="o_tmp", bufs=4) as otp:
                wo_sb = wop.tile([128, DT, D], BF16)
                bo_sb = wop.tile([128, DT], F32)
                nc.sync.dma_start(wo_sb[:], wo[:])
                nc.sync.dma_start(bo_sb[:], bo[:])
                xq_ts = []
                for t in range(QT):
                    xq_t = wop.tile([128, D], F32, tag=f"xq{t}")
                    nc.sync.dma_start(xq_t[:], xq[t * 128:(t + 1) * 128, :])
                    xq_ts.append(xq_t)
                for m in range(DT):
                    ps = ops.tile([128, 512], F32, tag="o_ps")
                    for k in range(DT):
                        nc.tensor.matmul(
                            ps[:], lhsT=wo_sb[:, k, m * 128:(m + 1) * 128],
                            rhs=attnT[:, k, :],
                            start=(k == 0), stop=(k == DT - 1))
                    yt = otp.tile([128, 512], BF16, tag="yt")
                    nc.vector.tensor_scalar_add(yt[:], ps[:], bo_sb[:, m:m + 1])
                    for t in range(QT):
                        tp = ops.tile([128, 128], BF16, tag="tp_ps")
                        nc.tensor.transpose(
                            tp[:], yt[:, t * 128:(t + 1) * 128], ident[:])
                        nc.vector.tensor_tensor(
                            x1[:, t, m * 128:(m + 1) * 128], tp[:],
                            xq_ts[t][:, m * 128:(m + 1) * 128], OP.add)

            # ---------- Phase 6: LN2 + FFN -----------------------------------
            with tc.tile_pool(name="ffn_tmp", bufs=3) as fp, \
                 tc.tile_pool(name="ffn_w", bufs=6) as fwp, \
                 tc.tile_pool(name="ffn_ps", bufs=4, space="PSUM") as fps, \
                 tc.tile_pool(name="h_pool", bufs=1) as hp:
                x2T = hp.tile([128, DT, TQ], BF16)
                for t in range(QT):
                    mean, rstd = ln_stats(x1[:, t, :])
                    x2_t = fp.tile([128, D], BF16, tag="x2_t")
                    ln_apply(x2_t[:], x1[:, t, :], mean, rstd)
                    for f in range(DT):
                        tp = fps.tile([128, 128], BF16, tag="tp2_ps")
                        nc.tensor.transpose(
                            tp[:], x2_t[:, f * 128:(f + 1) * 128], ident[:])
                        nc.any.tensor_copy(
                            x2T[:, f, t * 128:(t + 1) * 128], tp[:])

                b1_sb = hp.tile([128, FT], F32)
                b2_sb = hp.tile([128, DT], F32)
                nc.sync.dma_start(b1_sb[:], b1[:])
                nc.sync.dma_start(b2_sb[:], b2[:])

                h_sb = hp.tile([128, FT, TQ], BF16)
                for m in range(FT):
                    w1_m = fwp.tile([128, DT, 128], BF16, tag="w1_m")
                    nc.sync.dma_start(w1_m[:], w1[m])
                    ps = fps.tile([128, 512], F32, tag="f_ps")
                    for k in range(DT):
                        nc.tensor.matmul(
                            ps[:], lhsT=w1_m[:, k, :], rhs=x2T[:, k, :],
                            start=(k == 0), stop=(k == DT - 1))
                    nc.vector.tensor_scalar(
                        out=h_sb[:, m, :], in0=ps[:],
                        scalar1=b1_sb[:, m:m + 1], scalar2=0.0,
                        op0=OP.add, op1=OP.max)
                    nc.scalar.activation(
                        out=h_sb[:, m, :], in_=h_sb[:, m, :], func=AF.Square)
                out_ts = []
                for t in range(QT):
                    out_t = hp.tile([128, D], F32, tag=f"out{t}")
                    out_ts.append(out_t)
                for m in range(DT):
                    w2_m = fwp.tile([128, FT, 128], BF16, tag="w2_m")
                    nc.sync.dma_start(w2_m[:], w2[m])
                    ps = fps.tile([128, 512], F32, tag="f_ps")
                    for k in range(FT):
                        nc.tensor.matmul(
                            ps[:], lhsT=w2_m[:, k, :], rhs=h_sb[:, k, :],
                            start=(k == 0), stop=(k == FT - 1))
                    y2t = fp.tile([128, 512], BF16, tag="y2t")
                    nc.vector.tensor_scalar_add(y2t[:], ps[:], b2_sb[:, m:m + 1])
                    for t in range(QT):
                        tp = fps.tile([128, 128], BF16, tag="tp2_ps")
                        nc.tensor.transpose(
                            tp[:], y2t[:, t * 128:(t + 1) * 128], ident[:])
                        nc.vector.tensor_tensor(
                            out_ts[t][:, m * 128:(m + 1) * 128], tp[:],
                            x1[:, t, m * 128:(m + 1) * 128], OP.add)
                for t in range(QT):
                    nc.sync.dma_start(yout[t * 128:(t + 1) * 128, :], out_ts[t][:])

    nc.compile()
    return nc


def make_inputs(inputs, core):
    """Build the per-core input map from full inputs. core = 2*b + j."""
    bf = ml_dtypes.bfloat16
    b, j = core // 2, core % 2
    x = np.asarray(inputs["x"], np.float32)
    memory = np.asarray(inputs["memory"], np.float32)
    nmr = np.asarray(inputs["nmr"], np.float32)
    g1 = np.asarray(inputs["ln1_g"], np.float32)
    b1n = np.asarray(inputs["ln1_b"], np.float32)
    g2 = np.asarray(inputs["ln2_g"], np.float32)
    b2n = np.asarray(inputs["ln2_b"], np.float32)

    def pack_kd(w):
        # [D, n] -> [128, DT, n] with w[k*128+p, c] at [p, k, c]
        n = w.shape[1]
        return np.ascontiguousarray(w.reshape(DT, 128, n).transpose(1, 0, 2))

    def fold1(w, bias):
        wf = np.asarray(w, np.float32)
        bb = np.asarray(bias, np.float32)
        return pack_kd((wf * g1[:, None]).astype(bf)), \
            (bb + b1n @ wf).astype(np.float32)

    wq_, bq_ = fold1(inputs["Wq"], inputs["bq"])
    wk_, bk_ = fold1(inputs["Wk"], inputs["bk"])
    wv_, bv_ = fold1(inputs["Wv"], inputs["bv"])
    w1f = np.asarray(inputs["W1"], np.float32)
    w1b = (w1f * g2[:, None]).astype(bf)
    # [D, FFN] -> [FT, 128, DT, 128]
    w1_ = np.ascontiguousarray(
        w1b.reshape(DT, 128, FT, 128).transpose(2, 1, 0, 3))
    b1_ = (np.asarray(inputs["b1"], np.float32) + b2n @ w1f).astype(np.float32)
    wo_ = pack_kd(np.asarray(inputs["Wo"], np.float32).astype(bf))
    bo_ = np.asarray(inputs["bo"], np.float32)
    w2b = np.asarray(inputs["W2"], np.float32).astype(bf)
    # [FFN, D] -> [DT, 128, FT, 128]
    w2_ = np.ascontiguousarray(
        w2b.reshape(FT, 128, DT, 128).transpose(2, 1, 0, 3))
    b2_ = np.asarray(inputs["b2"], np.float32)

    xin = np.zeros((LP, D), np.float32)
    xin[:M] = memory[b]
    xin[M:PREF] = nmr[b]
    xin[PAD_PREF:] = x[b]
    qg = np.concatenate([g * 64 + np.arange(64) for g in QPERM[j]])
    xq = np.ascontiguousarray(x[b][qg])

    # rope tables (feature-major rows; rows r%64 in [0,32) are rope dims)
    r = np.arange(128)
    d_loc = r % 64
    is_rope = d_loc < ROT
    inv_freq = 1.0 / (10000.0 ** (np.arange(0, ROT, 2, dtype=np.float32) / ROT))
    freq_row = np.where(is_rope, inv_freq[(d_loc % 16)], 0.0)   # [128]

    pos_k = np.arange(LP, dtype=np.float32)
    pos_k[PREF:PAD_PREF] = 0.0
    pos_k[PAD_PREF:] = PREF + np.arange(T)
    pos_q = (PREF + qg).astype(np.float32)

    sgn = np.where((d_loc % 32) < 16, -1.0, 1.0)  # rotate-half sign on sin

    def tables(pos):
        ang = freq_row[:, None] * pos[None, :]
        cos = np.where(is_rope[:, None], np.cos(ang), 1.0).astype(bf)
        sin = np.where(is_rope[:, None], sgn[:, None] * np.sin(ang), 0.0).astype(bf)
        return np.ascontiguousarray(cos), np.ascontiguousarray(sin)

    cosk_, sink_ = tables(pos_k)
    cosq_, sinq_ = tables(pos_q)

    # mask [128, KK, TQ] (q columns follow the permuted block layout)
    key = (np.arange(128)[:, None] + 128 * np.arange(KK)[None, :])  # [128, KK]
    mask = np.zeros((128, KK, TQ), np.float32)
    prefix_ok = np.broadcast_to((key < PREF)[:, :, None], mask.shape)
    tk = key - PAD_PREF
    causal_ok = (key >= PAD_PREF)[:, :, None] & (tk[:, :, None] <= qg[None, None, :])
    mask[prefix_ok | causal_ok] = 1.0

    def bias_p(bias, nt):
        return np.ascontiguousarray(bias.reshape(nt, 128).T).astype(np.float32)

    return {
        "xin": xin, "xq": xq,
        "wq": wq_, "wk": wk_, "wv": wv_, "wo": wo_, "w1": w1_, "w2": w2_,
        "bq": bias_p(bq_, DT), "bk": bias_p(bk_, DT),
        "bv_r": bv_.reshape(1, D).astype(bf),
        "bo": bias_p(bo_, DT), "b1": bias_p(b1_, FT), "b2": bias_p(b2_, DT),
        "cosq": cosq_, "sinq": sinq_, "cosk": cosk_, "sink": sink_,
        "maskt": mask.astype(bf),
    }


_NC_CACHE = {}


def get_nc():
    if "nc" not in _NC_CACHE:
        _NC_CACHE["nc"] = build_kernel()
    return _NC_CACHE["nc"]


def kernel(**inputs) -> np.ndarray:
    from concourse.bass_utils import run_bass_kernel_spmd
    nc = get_nc()
    in_maps = [make_inputs(inputs, c) for c in range(8)]
    res = run_bass_kernel_spmd(nc, in_maps, list(range(8)))
    out = np.zeros((B, T, D), np.float32)
    for c in range(8):
        b, j = c // 2, c % 2
        qg = np.concatenate([g * 64 + np.arange(64) for g in QPERM[j]])
        out[b, qg] = res.results[c]["y"]
    return out


if __name__ == "__main__":
    nc = build_kernel()
    print("built ok")



# revision 33
# speedup vs baseline: 202.7005x; 202.7005x over previous
"""Trainium2 Bass kernel for nn_DecoderPromptLayerWithNMR.

Sharding: 8 fully-independent shards, core = (batch b in 0..3, query-half j in 0..1).
Each core computes the full layer for 512 target queries of one batch element.
No collectives. Feature-major attention with softmax-denominator via ones-column.

The attention-side GEMMs (Q/K/V/O projections and probs x V, with key tiles
paired) run in fp8e4 DoubleRow perf mode (~1.74x bf16 on HW); this side is
accuracy-free (3.6e-3 vs 3.1e-3 all-bf16). Weights are pre-scaled by SCL=32 on
the host so sigma~0.02 weights land in fp8e4's normal range; the inverse scale
is folded into existing ops (exp scale for scores, out-proj bias step), so
descaling costs zero extra instructions. The squared-relu FFN stays bf16: fp8
there alone costs ~3e-2 max-rel-err (squaring doubles the ~3% fp8 dot-product
noise, which does not average out), and hi/lo-compensated fp8 costs more than
bf16 at the measured DoubleRow speed. Elementwise work is spread across
DVE/ACT/Pool with PSUM consumers kept off Pool (no PSUM access from GPSIMD).
"""
import sys
import os

sys.path.insert(0, "/opt/trn_rl_repo")

import numpy as np
import ml_dtypes

import concourse.bass as bass
from concourse import bacc, mybir
from concourse.tile import TileContext

F32 = mybir.dt.float32
BF16 = mybir.dt.bfloat16
F8 = mybir.dt.float8e4
AF = mybir.ActivationFunctionType
OP = mybir.AluOpType
DR = mybir.MatmulPerfMode.DoubleRow

# Problem dims
D = 1024
H = 16
DH = 64
ROT = 32
FFN = 4096
B, T, M, N = 4, 1024, 128, 64
PREF = M + N            # 192
PAD_PREF = 256          # padded prefix (64 zero rows, masked)
LP = PAD_PREF + T       # 1280 padded key length
KK = LP // 128          # 10 key tiles
TQ = T // 2             # 512 queries per core
QT = TQ // 128          # 4 query token tiles
DT = D // 128           # 8 feature tiles
DT2 = DT // 2           # 4 DoubleRow k-pair steps
FT = FFN // 128         # 32 ffn tiles
FT2 = FT // 2
EPS = 1e-5
SCL = 32.0              # fp8 weight pre-scale
# per-key-tile active query columns (q-blocks sorted by descending causal
# extent per core; envelope over both core parities)
NACT = [512, 512, 512, 448, 384, 320, 256, 192, 128, 64]
# 64-query blocks, one of each causal extent per parity (identical multisets)
QPERM = {0: [14, 12, 10, 8, 6, 4, 2, 0], 1: [15, 13, 11, 9, 7, 5, 3, 1]}
VS = DH + 2             # V row stride (64 V + 1 ones + 1 pad, keeps 4B align)


def build_kernel(reps=1):
    """Build the per-core program. ``reps`` wraps the whole body in a
    hardware For_i loop executing it end-to-end that many times (used by
    test.py to measure true per-execution device time via the wall-clock
    slope over reps, which cancels the ~84ms axon tunnel RTT)."""
    nc = bacc.Bacc(None, target_bir_lowering=False)

    xin = nc.declare_dram_parameter("xin", [LP, D], F32, isOutput=False)
    xq = nc.declare_dram_parameter("xq", [TQ, D], F32, isOutput=False)
    # packed weights: w*[p, k, n] = W[k*128+p, n] (fp8, pre-scaled by SCL)
    wq = nc.declare_dram_parameter("wq", [128, DT, D], F8, isOutput=False)
    wk = nc.declare_dram_parameter("wk", [128, DT, D], F8, isOutput=False)
    wv = nc.declare_dram_parameter("wv", [128, DT, D], F8, isOutput=False)
    wo = nc.declare_dram_parameter("wo", [128, DT, D], F8, isOutput=False)
    # w1p[m, p, k, c] = W1[k*128+p, m*128+c]; w2p[m, p, k, c] = W2[k*128+p, m*128+c]
    w1 = nc.declare_dram_parameter("w1", [FT, 128, DT, 128], BF16, isOutput=False)
    w2 = nc.declare_dram_parameter("w2", [DT, 128, FT, 128], BF16, isOutput=False)
    bq = nc.declare_dram_parameter("bq", [128, DT], F32, isOutput=False)
    bk = nc.declare_dram_parameter("bk", [128, DT], F32, isOutput=False)
    bv_r = nc.declare_dram_parameter("bv_r", [1, D], BF16, isOutput=False)
    bo = nc.declare_dram_parameter("bo", [128, DT], F32, isOutput=False)
    b1 = nc.declare_dram_parameter("b1", [128, FT], F32, isOutput=False)
    b2 = nc.declare_dram_parameter("b2", [128, DT], F32, isOutput=False)
    cosq = nc.declare_dram_parameter("cosq", [128, TQ], BF16, isOutput=False)
    sinq = nc.declare_dram_parameter("sinq", [128, TQ], BF16, isOutput=False)
    cosk = nc.declare_dram_parameter("cosk", [128, LP], BF16, isOutput=False)
    sink = nc.declare_dram_parameter("sink", [128, LP], BF16, isOutput=False)
    maskt = nc.declare_dram_parameter("maskt", [128, KK, TQ], BF16, isOutput=False)
    yout = nc.declare_dram_parameter("y", [TQ, D], F32, isOutput=True)

    from contextlib import nullcontext
    with TileContext(nc) as tc:
        with (tc.For_i(0, reps) if reps > 1 else nullcontext()), \
             tc.tile_pool(name="persist", bufs=1) as persist, \
             tc.tile_pool(name="stats", bufs=10) as statsp:

            eps_t = persist.tile([128, 1], F32)
            nc.vector.memset(eps_t, EPS)
            ones_t = persist.tile([1, 128], BF16)
            nc.vector.memset(ones_t, 1.0)
            ident = persist.tile([128, 128], BF16)
            from concourse.masks import make_identity
            make_identity(nc, ident[:])

            v_sb = persist.tile([128, KK, H, VS], F8)  # V token-major + ones
            w2pf = persist.tile([128, 2, FT, 128], BF16)  # early w2[0:2] prefetch
            attnT = persist.tile([128, DT, TQ], F8)      # normalized attn^T (x SCL)
            x1 = persist.tile([128, QT, D], F32)         # attn-out + residual
            mask_sb = persist.tile([128, KK, TQ], BF16)

            nc.vector.memset(v_sb[:, :, :, DH:DH + 1], 1.0)

            def ln_stats(src_ap):
                """src_ap: [128,1024] fp32 -> (mean, rstd) [128,1] each."""
                st = statsp.tile([128, 2, 6], F32, tag="bn")
                nc.vector.bn_stats(out=st[:, 0, :], in_=src_ap[:, 0:512])
                nc.vector.bn_stats(out=st[:, 1, :], in_=src_ap[:, 512:1024])
                mv = statsp.tile([128, 2], F32, tag="mv")
                nc.vector.bn_aggr(out=mv[:], in_=st[:])
                rstd = statsp.tile([128, 1], F32, tag="rstd")
                nc.scalar.activation(out=rstd[:], in_=mv[:, 1:2],
                                     func=AF.Sqrt, bias=eps_t[:], scale=1.0)
                nc.vector.reciprocal(out=rstd[:], in_=rstd[:])
                return mv[:, 0:1], rstd

            def ln_apply(dst_ap, src_ap, mean, rstd):
                nmr_t = statsp.tile([128, 1], F32, tag="nmr")
                nc.vector.tensor_tensor(nmr_t[:], mean, rstd[:], OP.mult)
                nc.vector.tensor_scalar_mul(nmr_t[:], nmr_t[:], -1.0)
                nc.scalar.activation(out=dst_ap, in_=src_ap, func=AF.Identity,
                                     bias=nmr_t[:], scale=rstd[:])

            with tc.tile_pool(name="pa", bufs=1) as pa:
                # xaT split in two halves so PE work can start before LN1 ends
                xaT_a = pa.tile([128, DT, 512], F8)   # tokens 0..511
                xaT_b = pa.tile([128, DT, LP - 512], F8)  # tokens 512..1279
                xqT = pa.tile([128, DT, TQ], F8)      # LN1(xq)^T
                cosq_sb = pa.tile([128, TQ], BF16)
                sinq_sb = pa.tile([128, TQ], BF16)
                cosk_sb = pa.tile([128, LP], BF16)
                sink_sb = pa.tile([128, LP], BF16)

                def xa2(j, t0, t1):
                    # feature-major LN1(xin), k-pair j, tokens [t0, t1)
                    if t1 <= 512:
                        return xaT_a[:, 2 * j:2 * j + 2, t0:t1]
                    return xaT_b[:, 2 * j:2 * j + 2, t0 - 512:t1 - 512]

                from contextlib import ExitStack
                _wstack = ExitStack()
                wp = _wstack.enter_context(tc.tile_pool(name="w_p", bufs=1))
                wq_sb = wp.tile([128, DT, D], F8, tag="wq")
                wk_sb = wp.tile([128, DT, D], F8, tag="wk")
                wv_sb = wp.tile([128, DT, D], F8, tag="wv")
                bias_sb = wp.tile([128, 2 * DT], F32, tag="bias")
                bvr_sb = wp.tile([1, D], BF16, tag="bvr")

                # ---------- Phase 1+2: LN1 + transposes, xq first ------------
                with tc.tile_pool(name="ln", bufs=6) as lnp:

                    with tc.tile_pool(name="tr_ps", bufs=8, space="PSUM") as trps:
                        def ln_rows_t(src, t0, t1, dstT, dt0):
                            # LN + PE-transpose into feature-major dstT;
                            # half-column loads so bn_stats starts earlier
                            for t in range(t0, t1):
                                x_t = lnp.tile([128, D], F32, tag="ln_in")
                                nc.sync.dma_start(
                                    x_t[:, 0:512],
                                    src[t * 128:(t + 1) * 128, 0:512])
                                nc.sync.dma_start(
                                    x_t[:, 512:1024],
                                    src[t * 128:(t + 1) * 128, 512:1024])
                                mean, rstd = ln_stats(x_t[:])
                                xa_t = lnp.tile([128, D], BF16, tag="ln_out")
                                ln_apply(xa_t[:], x_t[:], mean, rstd)
                                d = t - dt0
                                for f in range(DT):
                                    tp = trps.tile([128, 128], BF16, tag="tr")
                                    nc.tensor.transpose(
                                        tp[:], xa_t[:, f * 128:(f + 1) * 128],
                                        ident[:])
                                    nc.any.tensor_copy(
                                        dstT[:, f, d * 128:(d + 1) * 128], tp[:])

                        ln_rows_t(xq, 0, 1, xqT, 0)
                        nc.sync.dma_start(wq_sb[:], wq[:])
                        nc.sync.dma_start(bias_sb[:, 0:DT], bk[:])
                        nc.sync.dma_start(bias_sb[:, DT:2 * DT], bq[:])
                        ln_rows_t(xq, 1, QT, xqT, 0)
                        nc.sync.dma_start(cosq_sb[:], cosq[:])
                        nc.sync.dma_start(sinq_sb[:], sinq[:])
                        nc.sync.dma_start(cosk_sb[:], cosk[:])
                        nc.sync.dma_start(sink_sb[:], sink[:])
                        ln_rows_t(xin, 0, 4, xaT_a, 0)
                        nc.sync.dma_start(wv_sb[:], wv[:])
                        nc.sync.dma_start(wk_sb[:], wk[:])
                        nc.sync.dma_start(bvr_sb[:], bv_r[:])
                        ln_rows_t(xin, 4, KK, xaT_b, 4)

                def rope(dst_ap, ntok, cos_sb, sin_sb, pool, rot_eng):
                    # sin table carries the rotate-half sign; shifts via DMA.
                    # ``rot_eng`` runs the rot*sin product (Pool for K, so it
                    # overlaps the dst*cos on DVE); dead-row zeroing is on the
                    # idle Pool engine.
                    rot = pool.tile([128, ntok], BF16, tag="rot")
                    nc.gpsimd.memset(rot[32:64, :], 0.0)
                    nc.gpsimd.memset(rot[96:128, :], 0.0)
                    nc.sync.dma_start(rot[0:16, :], dst_ap[16:32, :])
                    nc.sync.dma_start(rot[16:32, :], dst_ap[0:16, :])
                    nc.sync.dma_start(rot[64:80, :], dst_ap[80:96, :])
                    nc.sync.dma_start(rot[80:96, :], dst_ap[64:80, :])
                    rot_eng.tensor_tensor(rot[:], rot[:], sin_sb[:, :ntok], OP.mult)
                    nc.vector.tensor_tensor(dst_ap, dst_ap, cos_sb[:, :ntok], OP.mult)
                    nc.vector.tensor_tensor(dst_ap, dst_ap, rot[:], OP.add)

                # ---------- Phase 3a: Q proj, then V (token-major) -----------
                qTs = []
                with tc.tile_pool(name="v_ps", bufs=3, space="PSUM") as vps, \
                     tc.tile_pool(name="q_psp", bufs=2, space="PSUM") as qps:
                    for hg in range(H // 2):
                        qT_h = pa.tile([128, TQ], BF16, tag=f"qT{hg}")
                        ps = qps.tile([128, 512], F32, tag="q_ps")
                        for j in range(DT2):
                            nc.tensor.matmul(
                                ps[:],
                                lhsT=wq_sb[:, 2 * j:2 * j + 2,
                                           hg * 128:(hg + 1) * 128],
                                rhs=xqT[:, 2 * j:2 * j + 2, :],
                                start=(j == 0), stop=(j == DT2 - 1),
                                perf_mode=DR)
                        nc.vector.tensor_scalar_add(
                            qT_h[:], ps[:], bias_sb[:, DT + hg:DT + hg + 1])
                        rope(qT_h[:], TQ, cosq_sb, sinq_sb, statsp, nc.vector)
                        qTs.append(qT_h)
                    for kk in range(KK):
                        ps = vps.tile([128, 1024], F32, tag="v_ps")
                        for c0 in (0, 512):
                            for j in range(DT2):
                                nc.tensor.matmul(
                                    ps[:, c0:c0 + 512],
                                    lhsT=xa2(j, kk * 128, (kk + 1) * 128),
                                    rhs=wv_sb[:, 2 * j:2 * j + 2, c0:c0 + 512],
                                    start=(j == 0), stop=False, perf_mode=DR)
                            nc.tensor.matmul(
                                ps[:, c0:c0 + 512],
                                lhsT=ones_t[:],
                                rhs=bvr_sb[0:1, c0:c0 + 512],
                                start=False, stop=True)
                        nc.scalar.copy(
                            out=v_sb[:, kk, :, 0:DH],
                            in_=ps[:].rearrange("p (h d) -> p h d", h=H))
                        if kk == 1:
                            # zero V (and the denominator ones-column) for the
                            # 64 zero-pad prefix keys so no mask is needed for
                            # key tiles 0/1 (tile 0 is fully valid).
                            nc.gpsimd.memset(v_sb[64:128, 1, :, :], 0.0)

                # ---------- Phase 3b/4: per head-group K proj + attention ----
                with tc.tile_pool(name="kq_sb", bufs=3) as kqsb, \
                     tc.tile_pool(name="kq_ps", bufs=2, space="PSUM") as pps, \
                     tc.tile_pool(name="sc_ps", bufs=2, space="PSUM") as scps, \
                     tc.tile_pool(name="at_ps", bufs=2, space="PSUM") as atps, \
                     tc.tile_pool(name="att_tmp", bufs=4) as atp:
                    nc.sync.dma_start(mask_sb[:], maskt[:])
                    nc.sync.dma_start(w2pf[:, 0], w2[0])
                    nc.sync.dma_start(w2pf[:, 1], w2[1])
                    for hg in range(H // 2):
                        qT_h = qTs[hg]
                        # K^T tile for heads 2hg, 2hg+1
                        kT_h = kqsb.tile([128, LP], BF16, tag="kT")
                        for c0 in range(0, LP, 512):
                            cw = min(512, LP - c0)
                            ps = pps.tile([128, 512], F32, tag="kq_ps")
                            for j in range(DT2):
                                nc.tensor.matmul(
                                    ps[:, :cw],
                                    lhsT=wk_sb[:, 2 * j:2 * j + 2,
                                               hg * 128:(hg + 1) * 128],
                                    rhs=xa2(j, c0, c0 + cw),
                                    start=(j == 0), stop=(j == DT2 - 1),
                                    perf_mode=DR)
                            nc.vector.tensor_scalar_add(
                                kT_h[:, c0:c0 + cw], ps[:, :cw],
                                bias_sb[:, hg:hg + 1])
                        rope(kT_h[:], LP, cosk_sb, sink_sb, atp, nc.vector)

                        # attention for this head pair; key tiles are
                        # processed in pairs so probs x V runs as one fp8
                        # DoubleRow matmul per pair (contraction 256 keys).
                        ap0 = atps.tile([DH + 1, TQ], F32, tag="ap")
                        ap1 = atps.tile([DH + 1, TQ], F32, tag="ap")
                        aps = (ap0, ap1)
                        for kp in range(KK // 2):
                            nap = NACT[2 * kp]
                            # pexp layout: [keys, pair-member r, parity i, col]
                            pexp = atp.tile([128, 2, 2, 512], F8, tag="pexp")
                            for r in range(2):
                                kk = 2 * kp + r
                                na = NACT[kk]
                                sp = scps.tile([128, 1024], F32, tag="scores")
                                for i in range(2):
                                    r0 = 64 * i
                                    nc.tensor.matmul(
                                        sp[:, i * 512:i * 512 + na],
                                        lhsT=kT_h[r0:r0 + 64,
                                                  kk * 128:(kk + 1) * 128],
                                        rhs=qT_h[r0:r0 + 64, 0:na],
                                        start=True, stop=True)
                                if na == TQ:
                                    nc.scalar.activation(
                                        out=pexp[:, r].rearrange(
                                            "p a b -> p (a b)"),
                                        in_=sp[:], func=AF.Exp,
                                        scale=0.125 / (SCL * SCL))
                                else:
                                    nc.scalar.activation(
                                        out=pexp[:, r, :, 0:na],
                                        in_=sp[:].rearrange(
                                            "p (a b) -> p a b", a=2)[:, :, 0:na],
                                        func=AF.Exp, scale=0.125 / (SCL * SCL))
                                if r == 1 and na < nap:
                                    # exp only wrote cols [0, na); zero the
                                    # tail up to the pair width so the paired
                                    # matmul never reads uninitialized SBUF
                                    # (mask-multiplying garbage risks NaN).
                                    nc.gpsimd.memset(
                                        pexp[:, r, :, na:nap], 0.0)
                                if kk >= 2:
                                    # mask the partial boundary columns
                                    # (earlier columns are fully valid, tiles
                                    # 0/1 are handled by zeroed pad-V rows).
                                    na0 = max(0, na - 128)
                                    nc.gpsimd.tensor_tensor(
                                        pexp[:, r, :, na0:na],
                                        pexp[:, r, :, na0:na],
                                        mask_sb[:, kk, None, na0:na]
                                        .to_broadcast([128, 2, na - na0]),
                                        OP.mult)
                            for i in range(2):
                                nc.tensor.matmul(
                                    aps[i][:, 0:nap],
                                    lhsT=v_sb[:, 2 * kp:2 * kp + 2,
                                              2 * hg + i, 0:DH + 1],
                                    rhs=pexp[:, :, i, 0:nap],
                                    start=(kp == 0), stop=(kp == KK // 2 - 1),
                                    perf_mode=DR)
                        for i in range(2):
                            r_sb = atp.tile([1, TQ], F32, tag="recip")
                            nc.vector.reciprocal(r_sb[:], aps[i][DH:DH + 1, :])
                            bsb = atp.tile([64, TQ], F32, tag="bcs")
                            nc.gpsimd.partition_broadcast(bsb[:], r_sb[:])
                            nc.vector.tensor_tensor(
                                attnT[64 * i:64 * i + 64, hg, :],
                                aps[i][0:DH, :], bsb[:], OP.mult)

                _wstack.close()

            # ---------- Phase 5: out-proj + residual -------------------------
            with tc.tile_pool(name="wo_p", bufs=1) as wop, \
                 tc.tile_pool(name="o_ps", bufs=4, space="PSUM") as ops, \
                 tc.tile_pool(name="o_tmp", bufs=4) as otp:
                wo_sb = wop.tile([128, DT, D], F8)
                bo_sb = wop.tile([128, DT], F32)
                nc.sync.dma_start(wo_sb[:], wo[:])
                nc.sync.dma_start(bo_sb[:], bo[:])
                xq_ts = []
                for t in range(QT):
                    xq_t = wop.tile([128, D], F32, tag=f"xq{t}")
                    nc.sync.dma_start(xq_t[:], xq[t * 128:(t + 1) * 128, :])
                    xq_ts.append(xq_t)
                for m in range(DT):
                    ps = ops.tile([128, 512], F32, tag="o_ps")
                    for j in range(DT2):
                        nc.tensor.matmul(
                            ps[:],
                            lhsT=wo_sb[:, 2 * j:2 * j + 2, m * 128:(m + 1) * 128],
                            rhs=attnT[:, 2 * j:2 * j + 2, :],
                            start=(j == 0), stop=(j == DT2 - 1), perf_mode=DR)
                    yt = otp.tile([128, 512], BF16, tag="yt")
                    nc.vector.tensor_scalar(
                        out=yt[:], in0=ps[:], scalar1=1.0 / (SCL * SCL),
                        scalar2=bo_sb[:, m:m + 1], op0=OP.mult, op1=OP.add)
                    for t in range(QT):
                        tp = ops.tile([128, 128], BF16, tag="tp_ps")
                        nc.tensor.transpose(
                            tp[:], yt[:, t * 128:(t + 1) * 128], ident[:])
                        nc.vector.tensor_tensor(
                            x1[:, t, m * 128:(m + 1) * 128], tp[:],
                            xq_ts[t][:, m * 128:(m + 1) * 128], OP.add)

            # ---------- Phase 6: LN2 + FFN -----------------------------------
            with tc.tile_pool(name="ffn_tmp", bufs=3) as fp, \
                 tc.tile_pool(name="ffn_w", bufs=6) as fwp, \
                 tc.tile_pool(name="ffn_ps", bufs=3, space="PSUM") as fps, \
                 tc.tile_pool(name="ffn2_ps", bufs=2, space="PSUM") as f2ps, \
                 tc.tile_pool(name="h_pool", bufs=1) as hp:
                x2T = hp.tile([128, DT, TQ], BF16)
                for t in range(QT):
                    mean, rstd = ln_stats(x1[:, t, :])
                    x2_t = fp.tile([128, D], BF16, tag="x2_t")
                    ln_apply(x2_t[:], x1[:, t, :], mean, rstd)
                    for f in range(DT):
                        tp = fps.tile([128, 128], BF16, tag="tp2_ps")
                        nc.tensor.transpose(
                            tp[:], x2_t[:, f * 128:(f + 1) * 128], ident[:])
                        nc.any.tensor_copy(
                            x2T[:, f, t * 128:(t + 1) * 128], tp[:])

                b1_sb = hp.tile([128, FT], F32)
                b2_sb = hp.tile([128, DT], F32)
                nc.sync.dma_start(b1_sb[:], b1[:])
                nc.sync.dma_start(b2_sb[:], b2[:])

                h_sb = hp.tile([128, FT, TQ], BF16)
                for m in range(FT):
                    w1_m = fwp.tile([128, DT, 128], BF16, tag="w1_m")
                    nc.sync.dma_start(w1_m[:], w1[m])
                    ps = fps.tile([128, 512], F32, tag="f_ps")
                    for k in range(DT):
                        nc.tensor.matmul(
                            ps[:], lhsT=w1_m[:, k, :], rhs=x2T[:, k, :],
                            start=(k == 0), stop=(k == DT - 1))
                    hrelu = fp.tile([128, 512], BF16, tag="hrelu")
                    if m % 2 == 0:
                        # alternate engines so neither ACT nor DVE becomes the
                        # FFN1-phase limiter (PE is ~18us; keep peers under it)
                        nc.scalar.activation(
                            out=hrelu[:], in_=ps[:], func=AF.Relu,
                            bias=b1_sb[:, m:m + 1], scale=1.0)
                        nc.vector.scalar_tensor_tensor(
                            out=h_sb[:, m, :], in0=hrelu[:], scalar=1.0,
                            in1=hrelu[:], op0=OP.mult, op1=OP.mult)
                    else:
                        nc.vector.tensor_scalar(
                            out=hrelu[:], in0=ps[:],
                            scalar1=b1_sb[:, m:m + 1], scalar2=0.0,
                            op0=OP.add, op1=OP.max)
                        nc.gpsimd.tensor_tensor(
                            h_sb[:, m, :], hrelu[:], hrelu[:], OP.mult)
                out_ts = []
                for t in range(QT):
                    out_t = hp.tile([128, D], F32, tag=f"out{t}")
                    out_ts.append(out_t)
                for m in range(DT):
                    if m < 2:
                        w2_m = w2pf[:, m]
                    else:
                        w2_t = fwp.tile([128, FT, 128], BF16, tag="w2_m")
                        nc.sync.dma_start(w2_t[:], w2[m])
                        w2_m = w2_t[:]
                    ps = f2ps.tile([128, 512], F32, tag="f2_ps")
                    for k in range(FT):
                        nc.tensor.matmul(
                            ps[:], lhsT=w2_m[:, k, :], rhs=h_sb[:, k, :],
                            start=(k == 0), stop=(k == FT - 1))
                    y2t = fp.tile([128, 512], BF16, tag="y2t")
                    nc.vector.tensor_scalar_add(y2t[:], ps[:], b2_sb[:, m:m + 1])
                    for t in range(QT):
                        tp = fps.tile([128, 128], BF16, tag="tp2_ps")
                        nc.tensor.transpose(
                            tp[:], y2t[:, t * 128:(t + 1) * 128], ident[:])
                        nc.vector.tensor_tensor(
                            out_ts[t][:, m * 128:(m + 1) * 128], tp[:],
                            x1[:, t, m * 128:(m + 1) * 128], OP.add)
                    if m == DT // 2 - 1:
                        for t in range(QT):
                            nc.sync.dma_start(
                                yout[t * 128:(t + 1) * 128, 0:D // 2],
                                out_ts[t][:, 0:D // 2])
                for t in range(QT):
                    nc.sync.dma_start(
                        yout[t * 128:(t + 1) * 128, D // 2:D],
                        out_ts[t][:, D // 2:D])

    nc.compile()
    return nc


def make_inputs(inputs, core):
    """Build the per-core input map from full inputs. core = 2*b + j."""
    f8 = ml_dtypes.float8_e4m3
    bf = ml_dtypes.bfloat16
    b, j = core // 2, core % 2
    x = np.asarray(inputs["x"], np.float32)
    memory = np.asarray(inputs["memory"], np.float32)
    nmr = np.asarray(inputs["nmr"], np.float32)
    g1 = np.asarray(inputs["ln1_g"], np.float32)
    b1n = np.asarray(inputs["ln1_b"], np.float32)
    g2 = np.asarray(inputs["ln2_g"], np.float32)
    b2n = np.asarray(inputs["ln2_b"], np.float32)

    def pack_kd(w):
        # [D, n] -> [128, DT, n] with w[k*128+p, c] at [p, k, c]
        n = w.shape[1]
        return np.ascontiguousarray(w.reshape(DT, 128, n).transpose(1, 0, 2))

    def fold1(w, bias):
        # returns SCL-scaled fp8 weights and SCL-scaled bias (for q/k/v the
        # whole projection output carries a factor of SCL)
        wf = np.asarray(w, np.float32)
        bb = np.asarray(bias, np.float32)
        return pack_kd((wf * g1[:, None] * SCL).astype(f8)), \
            ((bb + b1n @ wf) * SCL).astype(np.float32)

    wq_, bq_ = fold1(inputs["Wq"], inputs["bq"])
    wk_, bk_ = fold1(inputs["Wk"], inputs["bk"])
    wv_, bv_ = fold1(inputs["Wv"], inputs["bv"])
    w1f = np.asarray(inputs["W1"], np.float32)
    w1b = (w1f * g2[:, None]).astype(bf)
    # [D, FFN] -> [FT, 128, DT, 128]
    w1_ = np.ascontiguousarray(
        w1b.reshape(DT, 128, FT, 128).transpose(2, 1, 0, 3))
    b1_ = (np.asarray(inputs["b1"], np.float32) + b2n @ w1f).astype(np.float32)
    wo_ = pack_kd((np.asarray(inputs["Wo"], np.float32) * SCL).astype(f8))
    bo_ = np.asarray(inputs["bo"], np.float32)
    w2b = np.asarray(inputs["W2"], np.float32).astype(bf)
    # [FFN, D] -> [DT, 128, FT, 128]
    w2_ = np.ascontiguousarray(
        w2b.reshape(FT, 128, DT, 128).transpose(2, 1, 0, 3))
    b2_ = np.asarray(inputs["b2"], np.float32)

    xin = np.zeros((LP, D), np.float32)
    xin[:M] = memory[b]
    xin[M:PREF] = nmr[b]
    xin[PAD_PREF:] = x[b]
    qg = np.concatenate([g * 64 + np.arange(64) for g in QPERM[j]])
    xq = np.ascontiguousarray(x[b][qg])

    # rope tables (feature-major rows; rows r%64 in [0,32) are rope dims)
    r = np.arange(128)
    d_loc = r % 64
    is_rope = d_loc < ROT
    inv_freq = 1.0 / (10000.0 ** (np.arange(0, ROT, 2, dtype=np.float32) / ROT))
    freq_row = np.where(is_rope, inv_freq[(d_loc % 16)], 0.0)   # [128]

    pos_k = np.arange(LP, dtype=np.float32)
    pos_k[PREF:PAD_PREF] = 0.0
    pos_k[PAD_PREF:] = PREF + np.arange(T)
    pos_q = (PREF + qg).astype(np.float32)

    sgn = np.where((d_loc % 32) < 16, -1.0, 1.0)  # rotate-half sign on sin

    def tables(pos):
        ang = freq_row[:, None] * pos[None, :]
        cos = np.where(is_rope[:, None], np.cos(ang), 1.0).astype(bf)
        sin = np.where(is_rope[:, None], sgn[:, None] * np.sin(ang), 0.0).astype(bf)
        return np.ascontiguousarray(cos), np.ascontiguousarray(sin)

    cosk_, sink_ = tables(pos_k)
    cosq_, sinq_ = tables(pos_q)

    # mask [128, KK, TQ] (q columns follow the permuted block layout)
    key = (np.arange(128)[:, None] + 128 * np.arange(KK)[None, :])  # [128, KK]
    mask = np.zeros((128, KK, TQ), np.float32)
    prefix_ok = np.broadcast_to((key < PREF)[:, :, None], mask.shape)
    tk = key - PAD_PREF
    causal_ok = (key >= PAD_PREF)[:, :, None] & (tk[:, :, None] <= qg[None, None, :])
    mask[prefix_ok | causal_ok] = 1.0

    def bias_p(bias, nt):
        return np.ascontiguousarray(bias.reshape(nt, 128).T).astype(np.float32)

    return {
        "xin": xin, "xq": xq,
        "wq": wq_, "wk": wk_, "wv": wv_, "wo": wo_, "w1": w1_, "w2": w2_,
        "bq": bias_p(bq_, DT), "bk": bias_p(bk_, DT),
        "bv_r": bv_.reshape(1, D).astype(bf),
        "bo": bias_p(bo_, DT), "b1": bias_p(b1_, FT), "b2": bias_p(b2_, DT),
        "cosq": cosq_, "sinq": sinq_, "cosk": cosk_, "sink": sink_,
        "maskt": mask.astype(bf),
    }


_NC_CACHE = {}


def get_nc(reps=1):
    if reps not in _NC_CACHE:
        _NC_CACHE[reps] = build_kernel(reps)
    return _NC_CACHE[reps]


def kernel(**inputs) -> np.ndarray:
    from concourse.bass_utils import run_bass_kernel_spmd
    nc = get_nc()
    in_maps = [make_inputs(inputs, c) for c in range(8)]
    res = run_bass_kernel_spmd(nc, in_maps, list(range(8)))
    out = np.zeros((B, T, D), np.float32)
    for c in range(8):
        b, j = c // 2, c % 2
        qg = np.concatenate([g * 64 + np.arange(64) for g in QPERM[j]])
        out[b, qg] = res.results[c]["y"]
    return out


if __name__ == "__main__":
    nc = build_kernel()
    print("built ok")
